# revision 15
# baseline (speedup 1.0000x reference)
"""Trainium2 Bass kernel for KNN-masked multi-head agent-agent attention.

Problem (per scene): N=1024 agents, D=256 model dim, H=4 heads, K=32 nearest
neighbours by distance. Full pipeline:
    top-K mask from distances -> additive bias (-d/50, -inf outside mask)
    -> MHA (shared in-proj, softmax, out-proj) -> residual + LayerNorm.

Sharding: data-parallel over the batch axis B=8 -> one scene per NeuronCore
(8 cores), no collectives. Each core runs the identical program (SPMD) on its
own scene; the host stacks per-core outputs.

Per-core algorithm:
  * selection: 4 rounds of (max8 + match_replace imm=-1e30) on nd=-d mark the
    exact top-32 multiset in-place: match_replace replaces the lowest-index
    occurrence of each of the 8 values per round, which reproduces
    jax.lax.top_k's index tie-breaking exactly for any tie multiplicity.
    Selected entries of sc equal -1e30 afterwards; everything else keeps nd.
    All nd conversions are hoisted up front so the DVE runs the 64 selection
    ops back-to-back; mask/bias arithmetic is offloaded to the Pool engine.
  * bias (negative-offset form, fp16): bias = 0.02*nd - 44*[not selected].
    Selected entries carry only 0.02*nd (full fp16 precision); masked entries
    sit near -44 where precision is irrelevant and exp underflows fp16 to 0.
    The natural-layout fp16 bias is transposed by the DMA xbar
    (dma_start_transpose) into per-key-block layout.
  * attention in transposed layout, interleaved with selection in 256-query
    chunks: S^T = K_h Q_h^T (f32r) + identity-matmul accumulation of the
    transposed fp16 bias, probs = exp(PSUM) in fp16 (one activation per
    4-key-block PSUM group), AV^T in fp16 with a ones-augmented V so the
    softmax denominator falls out of the same matmul; late normalization via
    a PE selector-matmul broadcast of the reciprocal denominators.
  * out-proj back to natural layout (fp16 weights), residual + LayerNorm
    epilogue (bn_stats on DVE, scale/shift on Pool).
"""

import os
import sys
import numpy as np

sys.path.insert(0, "/opt/trn_rl_repo")

import concourse.bass as bass
import concourse.tile as tile
from concourse import mybir
from concourse.masks import make_identity

f32 = mybir.dt.float32
f32r = mybir.dt.float32r
f16 = mybir.dt.float16
Alu = mybir.AluOpType
Act = mybir.ActivationFunctionType

N = 1024
D = 256
H = 4
HD = 64
NT = N // 128          # 8 query/token tiles
KB = N // 128          # 8 key blocks
D_REF = 50.0
LN_EPS = 1e-5
NEG_BIG = -1.0e30
MASK_M = -44.0         # additive mask for non-selected entries (exp -> 0)

MM_DT = f32r


def build_nc(K: int, split_waits: bool = True):
    nc = bass.Bass("TRN2", target_bir_lowering=False, debug=False)

    x_d = nc.dram_tensor("repr1", [N, D], f32, kind="ExternalInput").ap()
    d_d = nc.dram_tensor("distances", [N, N], f32, kind="ExternalInput").ap()
    wi_d = nc.dram_tensor("in_proj_w", [3 * D, D], f32, kind="ExternalInput").ap()
    bi_d = nc.dram_tensor("in_proj_b", [3 * D], f32, kind="ExternalInput").ap()
    wo_d = nc.dram_tensor("out_proj_w", [D, D], f32, kind="ExternalInput").ap()
    bo_d = nc.dram_tensor("out_proj_b", [D], f32, kind="ExternalInput").ap()
    g_d = nc.dram_tensor("ln_gamma", [D], f32, kind="ExternalInput").ap()
    be_d = nc.dram_tensor("ln_beta", [D], f32, kind="ExternalInput").ap()
    out_d = nc.dram_tensor("out", [N, D], f32, kind="ExternalOutput").ap()

    with tile.TileContext(nc) as tc:
        _emit(tc, K, x_d, d_d, wi_d, bi_d, wo_d, bo_d, g_d, be_d, out_d)
    if split_waits:
        _split_waits(nc)
    return nc


def _split_waits(nc, max_waits: int = 1):
    """Walrus codegen rejects instructions carrying more than one sync wait
    (e.g. transpose-matmul LDW structs and HWDGE DMA descriptors), and the
    DMA_DIRECT2D_XPOSE struct carries none at all. Move the excess waits onto
    engine NoOps issued immediately before — the sequencer stalls on those
    first, which is semantically identical."""
    k = 0
    for fn in nc.m.functions:
        for blk in fn.blocks:
            new = []
            for ins in blk.instructions:
                si = ins.sync_info
                mw = 0 if isinstance(ins, mybir.InstDmaTransposeAnt) else max_waits
                if si is not None and si.on_wait and len(si.on_wait) > mw:
                    waits = list(si.on_wait)
                    keep = waits[-mw:] if mw else []
                    for w in (waits[:-mw] if mw else waits):
                        nop = mybir.InstNoOp(
                            name=f"I-wsplit-{k}", engine=ins.engine)
                        nop.sync_info = mybir.SyncInfo(on_wait=[w], on_update=[])
                        new.append(nop)
                        k += 1
                    ins.sync_info = mybir.SyncInfo(
                        on_wait=keep, on_update=list(si.on_update))
                new.append(ins)
            blk.instructions[:] = new


def _bcast_dram_row(nc, dst, src_ap, offset, width):
    """DMA-replicate a [width] DRAM row into all 128 partitions of dst."""
    rep = bass.AP(
        tensor=src_ap.tensor,
        offset=src_ap.offset + offset,
        ap=[[0, 128], [1, width]],
    )
    nc.gpsimd.dma_start(out=dst, in_=rep)


def _emit(tc, K, x_d, d_d, wi_d, bi_d, wo_d, bo_d, g_d, be_d, out_d):
    from contextlib import ExitStack
    nc = tc.nc
    ctx = ExitStack()

    consts = ctx.enter_context(tc.tile_pool(name="consts", bufs=1))
    persist = ctx.enter_context(tc.tile_pool(name="persist", bufs=1))
    dstage = ctx.enter_context(tc.tile_pool(name="dstage", bufs=2))
    dpre = ctx.enter_context(tc.tile_pool(name="dpre", bufs=3))
    ndp = ctx.enter_context(tc.tile_pool(name="ndp", bufs=1))
    selp = ctx.enter_context(tc.tile_pool(name="selp", bufs=2))
    ptp = ctx.enter_context(tc.tile_pool(name="ptp", bufs=3))
    epi = ctx.enter_context(tc.tile_pool(name="epi", bufs=3))
    ps_s = ctx.enter_context(tc.tile_pool(name="ps_s", bufs=2, space="PSUM"))
    ps_av = ctx.enter_context(tc.tile_pool(name="ps_av", bufs=1, space="PSUM"))
    ps_tr = ctx.enter_context(tc.tile_pool(name="ps_tr", bufs=1, space="PSUM"))
    ps_o = ctx.enter_context(tc.tile_pool(name="ps_o", bufs=1, space="PSUM"))
    ps_rb = ctx.enter_context(tc.tile_pool(name="ps_rb", bufs=1, space="PSUM"))

    # ---------------- constants ----------------
    ident = consts.tile([128, 128], f32, name="ident")
    make_identity(nc, ident)
    identh = consts.tile([128, 128], f16, name="identh")
    nc.gpsimd.tensor_copy(identh, ident)
    # PE touches ident once so later transpose-matmuls (which can carry only
    # a single sync wait in walrus codegen) need no wait on producers.
    identwarm = ps_tr.tile([128, 128], f32, name="identwarm", tag="wtr")
    nc.tensor.matmul(identwarm, lhsT=ident, rhs=ident, is_transpose=True)

    epsc = consts.tile([128, 1], f32, name="epsc")
    nc.gpsimd.memset(epsc, LN_EPS)

    # head-half selector for the denominator broadcast matmul:
    # sel2[0, p] = [p < 64], sel2[1, p] = [p >= 64]
    sel2 = consts.tile([2, 128], f32, name="sel2")
    iota128 = consts.tile([2, 128], f32, name="iota128")
    nc.gpsimd.iota(iota128, pattern=[[1, 128]], base=0, channel_multiplier=0,
                   allow_small_or_imprecise_dtypes=True)
    nc.gpsimd.tensor_scalar(sel2[0:1, :], iota128[0:1, :], float(HD), None,
                            Alu.is_lt)
    nc.gpsimd.tensor_scalar(sel2[1:2, :], iota128[1:2, :], float(HD), None,
                            Alu.is_ge)

    # ------- distance prefetch (SP queue first) + upfront nd conversion ----
    nds = [ndp.tile([128, N], f32, name=f"nd{i}") for i in range(NT)]
    for i in range(NT):
        drow = dpre.tile([128, N], f32, name="drow", tag="drow")
        nc.sync.dma_start(out=drow, in_=d_d[i * 128:(i + 1) * 128, :])
        nc.scalar.activation(nds[i], drow, Act.Copy, scale=-1.0)  # nd = -d

    # ---------------- weights ----------------
    # W^T for in-proj: [256, 768] as 2 partition tiles of [128, 768]
    wt = [persist.tile([128, 3 * D], f32, name=f"wt{c}") for c in range(2)]
    for r in range(6):  # six [128, 256] row-tiles of in_proj_w
        wrow = dstage.tile([128, D], f32, name="wrow", tag="wrow")
        nc.sync.dma_start(out=wrow, in_=wi_d[r * 128:(r + 1) * 128, :])
        for c in range(2):
            pt = ps_tr.tile([128, 128], f32, name="wtr", tag="wtr")
            nc.tensor.matmul(pt, lhsT=wrow[:, c * 128:(c + 1) * 128], rhs=ident,
                             is_transpose=True)
            nc.gpsimd.tensor_copy(wt[c][:, r * 128:(r + 1) * 128].bitcast(f32r), pt)
    # fold the attention scale 1/8 into Wq^T (free cols 0..255 = Q features)
    for c in range(2):
        nc.gpsimd.tensor_scalar_mul(wt[c][:, 0:D].bitcast(f32r), wt[c][:, 0:D], 0.125)

    # Wo^T [256, 256] as 2 fp16 tiles [128, 256]
    wot = [persist.tile([128, D], f16, name=f"wot{c}") for c in range(2)]
    for r in range(2):
        worow = dstage.tile([128, D], f32, name="worow", tag="wrow")
        nc.sync.dma_start(out=worow, in_=wo_d[r * 128:(r + 1) * 128, :])
        for c in range(2):
            pt = ps_tr.tile([128, 128], f32, name="wotr", tag="wtr")
            nc.tensor.matmul(pt, lhsT=worow[:, c * 128:(c + 1) * 128], rhs=ident,
                             is_transpose=True)
            nc.gpsimd.tensor_copy(wot[c][:, r * 128:(r + 1) * 128], pt)

    # per-partition in-proj biases for the Q^T/K^T M-blocks (Q biases pre-scaled)
    bqk = []
    for mb in range(4):
        t = consts.tile([128, 1], f32, name=f"bqk{mb}")
        nc.sync.dma_start(out=t, in_=bi_d[mb * 128:(mb + 1) * 128].rearrange(
            "(p o) -> p o", o=1))
        if mb < 2:
            nc.gpsimd.tensor_scalar_mul(t, t, 0.125)
        bqk.append(t)

    bv_b = consts.tile([128, D], f32, name="bv_b")
    _bcast_dram_row(nc, bv_b, bi_d, 2 * D, D)
    bo_b = consts.tile([128, D], f32, name="bo_b")
    _bcast_dram_row(nc, bo_b, bo_d, 0, D)
    g_b = consts.tile([128, D], f32, name="g_b")
    _bcast_dram_row(nc, g_b, g_d, 0, D)
    be_b = consts.tile([128, D], f32, name="be_b")
    _bcast_dram_row(nc, be_b, be_d, 0, D)

    # ---------------- X, Xb, X^T ----------------
    xb = []  # residual + out-proj bias pre-added
    xt = [persist.tile([128, N], f32, name=f"xt{c}") for c in range(2)]
    for i in range(NT):
        xrow = dstage.tile([128, D], f32, name="xrow", tag="wrow")
        nc.sync.dma_start(out=xrow, in_=x_d[i * 128:(i + 1) * 128, :])
        for c in range(2):
            pt = ps_tr.tile([128, 128], f32, name="xtr", tag="wtr")
            nc.tensor.matmul(pt, lhsT=xrow[:, c * 128:(c + 1) * 128], rhs=ident,
                             is_transpose=True)
            nc.gpsimd.tensor_copy(xt[c][:, i * 128:(i + 1) * 128].bitcast(f32r), pt)
        t = persist.tile([128, D], f32, name=f"xb{i}")
        nc.gpsimd.tensor_tensor(t, xrow, bo_b, Alu.add)
        xb.append(t)

    # ---------------- Q^T, K^T, V ----------------
    qkt = [persist.tile([128, N], f32, name=f"qkt{mb}") for mb in range(4)]
    for mb in range(4):
        for qc in range(4):
            ps = ps_o.tile([128, D], f32, name="qk_ps", tag="ps_o")
            for c in range(2):
                nc.tensor.matmul(
                    ps,
                    lhsT=wt[c][:, mb * 128:(mb + 1) * 128].bitcast(MM_DT),
                    rhs=xt[c][:, qc * 256:(qc + 1) * 256].bitcast(MM_DT),
                    start=(c == 0), stop=(c == 1))
            nc.scalar.activation(qkt[mb][:, qc * 256:(qc + 1) * 256].bitcast(f32r),
                                 ps, Act.Identity, bias=bqk[mb])

    # V padded per head, fp16: [128, H, 65]; col 64 of each head slot is the
    # ones column that produces the softmax denominator in the AV matmul.
    vpad = [persist.tile([128, H, HD + 1], f16, name=f"vpad{kb}") for kb in range(KB)]
    ones4 = consts.tile([128, H], f16, name="ones4")
    nc.gpsimd.memset(ones4, 1.0)
    for kb in range(KB):
        nc.gpsimd.tensor_copy(
            vpad[kb][:, :, HD:HD + 1],
            ones4.rearrange("p (h o) -> p h o", o=1))
        ps = ps_o.tile([128, D], f32, name="v_ps", tag="ps_o")
        for c in range(2):
            nc.tensor.matmul(
                ps,
                lhsT=xt[c][:, kb * 128:(kb + 1) * 128].bitcast(MM_DT),
                rhs=wt[c][:, 2 * D:3 * D].bitcast(MM_DT),
                start=(c == 0), stop=(c == 1))
        nc.gpsimd.tensor_tensor(
            vpad[kb][:, :, 0:HD],
            ps.rearrange("p (h e) -> p h e", h=H),
            bv_b.rearrange("p (h e) -> p h e", h=H),
            Alu.add)

    # ---------------- selection + bias + attention, interleaved ----------
    # biasf[q, k] = 0.02 * nd - 44 * [k not in top-32(q)]   (fp16, natural)
    # bias_t[kk, kb, q] = biasf[q, kb*128 + kk]             (DMA-transposed)
    bias_t = persist.tile([128, KB, N], f16, name="bias_t")
    attnt = [persist.tile([128, N], f16, name=f"attnt{c}") for c in range(2)]
    den4 = persist.tile([4, N], f32, name="den4")
    rd2 = [persist.tile([2, N], f32, name=f"rd2{c}") for c in range(2)]

    def select_tile(i):
        nd = nds[i]
        m32 = selp.tile([128, 32], f32, name="m32", tag="m32")
        sc = selp.tile([128, N], f32, name="selsc", tag="selsc")
        nc.vector.max(m32[:, 0:8], nd)
        nc.vector.match_replace(sc, m32[:, 0:8], nd, NEG_BIG)
        nc.vector.max(m32[:, 8:16], sc)
        nc.vector.match_replace(sc, m32[:, 8:16], sc, NEG_BIG)
        nc.vector.max(m32[:, 16:24], sc)
        nc.vector.match_replace(sc, m32[:, 16:24], sc, NEG_BIG)
        nc.vector.max(m32[:, 24:32], sc)
        nc.vector.match_replace(sc, m32[:, 24:32], sc, NEG_BIG)
        # sc == NEG_BIG exactly marks the reference top-32 multiset.

        # m40 = -44 where NOT selected, 0 where selected  (Pool)
        m40 = selp.tile([128, N], f32, name="m40", tag="m40")
        nc.gpsimd.tensor_scalar(m40, sc, 0.5 * NEG_BIG, MASK_M,
                                Alu.is_gt, Alu.mult)
        # biasf = 0.02*nd + m40  (fp16 out, Pool)
        biasf = selp.tile([128, N], f16, name="biasf", tag="biasf")
        nc.gpsimd.scalar_tensor_tensor(
            out=biasf, in0=nd, scalar=1.0 / D_REF, in1=m40,
            op0=Alu.mult, op1=Alu.add)
        # transpose into bias_t columns i*128..(i+1)*128 (runs on DMA xbar)
        nc.sync.dma_start_transpose(
            out=bias_t[:, :, i * 128:(i + 1) * 128], in_=biasf)

    def attn_chunk(q0, QW):
        qs = slice(q0, q0 + QW)
        for h in range(H):
            qmb, kmb = h // 2, 2 + h // 2
            p0 = (h % 2) * HD
            pt_groups = []
            for g in range(2):      # 2 groups of 4 key blocks
                ps = ps_s.tile([128, 4, QW], f32, name="s_ps", tag="ps_s")
                for j in range(4):
                    kb = 4 * g + j
                    nc.tensor.matmul(
                        ps[:, j, :],
                        lhsT=qkt[kmb][p0:p0 + HD, kb * 128:(kb + 1) * 128].bitcast(MM_DT),
                        rhs=qkt[qmb][p0:p0 + HD, qs].bitcast(MM_DT),
                        start=True, stop=False)
                    nc.tensor.matmul(
                        ps[:, j, :], lhsT=identh, rhs=bias_t[:, kb, qs],
                        start=False, stop=True)
                ptg = ptp.tile([128, 4, QW], f16, name="pt", tag="pt")
                nc.scalar.activation(ptg, ps, Act.Exp)
                pt_groups.append(ptg)
            av = ps_av.tile([HD + 1, QW], f32, name="av_ps", tag="ps_av")
            for kb in range(KB):
                nc.tensor.matmul(
                    av,
                    lhsT=vpad[kb][:, h, :],
                    rhs=pt_groups[kb // 4][:, kb % 4, :],
                    start=(kb == 0), stop=(kb == KB - 1))
            nc.scalar.activation(
                attnt[h // 2][(h % 2) * HD:(h % 2) * HD + HD, qs],
                av[0:HD, :], Act.Copy)
            nc.gpsimd.tensor_copy(den4[h:h + 1, qs], av[HD:HD + 1, :])

        # ---- normalize + out-proj + LayerNorm epilogue for this chunk
        for c in range(2):
            nc.vector.reciprocal(rd2[c][:, qs], den4[2 * c:2 * c + 2, qs])
            rbp = ps_rb.tile([128, QW], f32, name="rb_ps", tag="ps_rb")
            nc.tensor.matmul(rbp, lhsT=sel2.bitcast(MM_DT),
                             rhs=rd2[c][:, qs].bitcast(MM_DT))
            rbh = epi.tile([128, QW], f16, name="rbh", tag="rbh")
            nc.scalar.activation(rbh, rbp, Act.Copy)
            nc.gpsimd.tensor_tensor(attnt[c][:, qs], attnt[c][:, qs], rbh,
                                    Alu.mult)

        for tb in range(q0 // 128, (q0 + QW) // 128):
            po = ps_o.tile([128, D], f32, name="o_ps", tag="ps_o")
            for c in range(2):
                nc.tensor.matmul(
                    po,
                    lhsT=attnt[c][:, tb * 128:(tb + 1) * 128],
                    rhs=wot[c],
                    start=(c == 0), stop=(c == 1))
            x = epi.tile([128, D], f32, name="x_epi", tag="x_epi")
            nc.gpsimd.tensor_tensor(x, po, xb[tb], Alu.add)
            st = epi.tile([128, 6], f32, name="st", tag="st")
            nc.vector.bn_stats(st, x)
            mv = epi.tile([128, 2], f32, name="mv", tag="mv")
            nc.vector.bn_aggr(mv, st)
            sd = epi.tile([128, 1], f32, name="sd", tag="sd")
            nc.scalar.activation(sd, mv[:, 1:2], Act.Sqrt, bias=epsc)
            rstd = epi.tile([128, 1], f32, name="rstd", tag="rstd")
            nc.vector.reciprocal(rstd, sd)
            xc = epi.tile([128, D], f32, name="xc_epi", tag="xc_epi")
            nc.gpsimd.tensor_scalar(xc, x, mv[:, 0:1], None, Alu.subtract)
            y = epi.tile([128, D], f32, name="y_epi", tag="y_epi")
            nc.vector.scalar_tensor_tensor(
                out=y, in0=g_b, scalar=rstd, in1=xc, op0=Alu.mult, op1=Alu.mult)
            nc.gpsimd.tensor_tensor(y, y, be_b, Alu.add)
            nc.scalar.dma_start(
                out=out_d[tb * 128:(tb + 1) * 128, :], in_=y)

    # interleave with a one-pair lag: chunk c is emitted after selection
    # tiles 2c+2, 2c+3, so the in-order DVE stream never waits on a chunk
    # pipeline that isn't finished yet.
    select_tile(0)
    select_tile(1)
    for ch in range(4):
        if 2 * ch + 3 < NT:
            select_tile(2 * ch + 2)
            select_tile(2 * ch + 3)
        attn_chunk(256 * ch, 256)

    ctx.close()


_NC_CACHE = {}


def _get_nc(K: int):
    if K not in _NC_CACHE:
        _NC_CACHE[K] = build_nc(K)
    return _NC_CACHE[K]


def kernel(**inputs) -> np.ndarray:
    from concourse.bass_utils import run_bass_kernel_spmd

    K = int(np.asarray(inputs["K"]))
    assert K == 32, f"kernel specialized for K=32, got {K}"
    B = inputs["repr1"].shape[0]
    nc = _get_nc(K)

    shared = {
        "in_proj_w": np.ascontiguousarray(inputs["in_proj_w"], np.float32),
        "in_proj_b": np.ascontiguousarray(inputs["in_proj_b"], np.float32),
        "out_proj_w": np.ascontiguousarray(inputs["out_proj_w"], np.float32),
        "out_proj_b": np.ascontiguousarray(inputs["out_proj_b"], np.float32),
        "ln_gamma": np.ascontiguousarray(inputs["ln_gamma"], np.float32),
        "ln_beta": np.ascontiguousarray(inputs["ln_beta"], np.float32),
    }
    in_maps = []
    for b in range(B):
        m = dict(shared)
        m["repr1"] = np.ascontiguousarray(inputs["repr1"][b], np.float32)
        m["distances"] = np.ascontiguousarray(inputs["distances"][b], np.float32)
        in_maps.append(m)

    res = run_bass_kernel_spmd(nc, in_maps, list(range(B)))
    out = np.stack([np.asarray(res.results[b]["out"]) for b in range(B)])
    return out.astype(np.float32)
